# revision 1
# baseline (speedup 1.0000x reference)
"""Trainium2 Bass/Tile kernel for nn_Apply2DTform: batched affine warp with
round-nearest bilinear sampling.

Sharding: pure data parallel, batch 32 -> 8 cores x 4 images each.

This hardware's indirect DMA honors exactly ONE offset per partition
(contiguous fetch of the dest-row size), so the gather is organized as:
  - Build a zero-padded image [514 x 514] in DRAM, then a row-pair
    interleaved copy P[(r*514+c)*2 + h] = img[r+h, c].  The 4 bilinear taps
    of a pixel are then 4 CONTIGUOUS floats at (x0c*514+y0c)*2.
  - One indirect-DMA descriptor per output pixel (128 pixels per call, one
    per partition; 512 calls per 128-row block).
  - Clip collisions (x1c==x0c or y1c==y0c) are handled by folding weights so
    the pair elements that don't match the reference get weight exactly 0.
  - Coordinates/weights replicate the jax reference's exact f32 op order;
    rounding via the (2^23+2^22) magic-add (ties-to-even like jnp.round).

kernel(**inputs): full (32,512,512,1)+(32,6) in -> full (32,512,512,1) out.
"""
import os
import sys

sys.path.insert(0, "/opt/trn_rl_repo")

import numpy as np

import concourse.bass as bass
import concourse.mybir as mybir
import concourse.tile as tile
from concourse.bass_utils import run_bass_kernel_spmd

f32 = mybir.dt.float32
i32 = mybir.dt.int32
A = mybir.AluOpType

N_CORES = 8
B_PER = 4
H = W = 512
PH, PW = 514, 514            # padded image rows x cols (rows 512/513 zero)
NP_IMG = PH * PW
NPP = (513 * 514) * 2 + 4    # pair image elements per image (+ slack)
MAGIC = 12582912.0           # 2^23 + 2^22

LAST_EXEC_NS = None
LAST_RESULTS = None
_LEGALIZE = True  # sim can't handle injected NoOps; set False for --sim

_AX_B64 = "eJwN1YdT1Hcax3EuxfNM4nkm8TxTbkGCaJAm0RX39zxIE2kqTaSJ1GUp4tLWpcTMJZPJJDdJ5pLJZJKbU4OKShQNKsr+ngcLGuwNDXZ671VEvOdf+H6fz/tlZbWDd7woYasXxbxjuoitpgu55LmZp6e2c8mUiaefFXDJs3yenszjkslcnnqaw0VPjTw1sY2LJrJ5anwrF41n8dRYJheNZfDUaDqbRw08OZLG5hE9Tw6nsnk4hSeHktk8lMQTg4lsGkzgiYEtbBqI54n+zWzqj+OJvlg29cXwRG805/dG8VjPJs7vieSx7o2c3x3BY13hnN8VxiOdoZzbGcIjHRs4t2M9j7Sv49z2YB5pC+LctkAeaQ1gY6s/D7WsZWOLHw81r2Fjsy8PNfmwscmbBxq9OLvRkweerObsJx488Bg5+zHwwCOFsx/peODhKs566M59D1Zy1gMt991fwVn3l3PfvY84654b9zQs44wGV+75w4Uz/nDmnrtOnHHXkXvuLOWMOw7cU/8hG+qXcNftxWy4bc9dtxax4ZYdd938gA03bbnjxkLW37DhjuvWrL+u4Y5r/2T9tfe54+p7rL/6LndceYdTrizgtsv/4JTL87nt0t855dI8brv4NqdcfItb6t7kpLq53PL73zjp9znccuGvnHRhNrecf4OTzr/OLbWvcULtLG469xdOODeTm87+mRPOzuCmM69ywplX+Mnplzn+9Ev8pOZPHF9jxTUfv6CakudUU/yM5GOJC8dJNY+SZfswWUyDZCnoJ0t+L1nyuqk6t5NO5rTTSWMrVW1rpqrsRqra+piqsh5SVeZ9OpHRQCfS79IxQz1Vpt2iSv0Nqky9RpUpV6gy+RL9llRHRxIv0JGEWqrYcpYq4k9TxWamijiVKmKr6XDMSTocfYJ+jTpG5Zt+o/LII1S+8TCVR/xK5eEH6WDYftofuo/2h+yhsg2/UNn6XVS27n9UFvxfKgv6ifYF/kj7An6gPf7fU+na/1Cp37dUuuZrKvX9N5X6fEm/eH9Bu7w+p12en9HO1f+inR6fkMfHJeRRbCYszCfYbiQoyCL3PAOtzEkh7bYE0m6NI21mFK1Ij6DlaSHkmhpMrsn+5JLoS85bPMl5M5BzrDs5RS8nx02u5LjRkZaEL6HFoXZkv8GG7Ne9T/ZBC2hRwDyyWzuXbNbMJhufWWTtNYM0q18ij5IpFc0jKuT3qIqxRdVlPVCXGW6rrimXVZeEc6pznEV1iqpUHSPK1aUhpapt8M/qQv/vVBvfr1Rrz09VjyKTCrl6VZcRqbon+anaGK1qF2av2gbOV228Z6oepgGLTn/doo2ssLj5fWPRZMdb5q/XWDTxNdUaD6tqKw+rVXPia1bNWa/RabPjdRq/b3S2kRU6rf66Tmca0Gm8Zyo2gfMV2zB7xTFGqzgn+SmuGZGKLlevQJFJ0Xh+qlj7fqXY+H+nLAr+WbEPKVUWR5QrjlGVilOcRXFOOKe4pFxWXA23lRVZDxStsUVZmd+jgHlEwZIpRd4ANF4zwNpnFtiumQ22a+eCbcA8WBS0AOTtQN4Q7EPtYHH4EnDY6AgOm1zBIXo5OMW6g7w9yB+Ac6IvuCT7g1tqMLilhYBbegSsyIwC+TuQPwRtTgqszDOAriALdNuNoCvMByw2g/w9yA2A3ALs9PwMdnt9Dru9v4DdPl+C3AzI7YDcEMgtQan/97A34AfYG/gj7A36CeTmQG4P5AZBbhHKQvbAgdB9cCBsPxwIPwhysyC3C3LDILcM5VHH4FD0CTgUcxIOxVaD3DzI7YNsAGQLUJFQC0cTL8DRpDo4mnwJZDMg2wHZEMiWoNJQD8fT78LxjAY4nnkfZHMg2wPZIMgWocrYCqdy2uFUbiecyusG2SzIdkE2DLJlsJhHgQrHgYqeAhU/A9k8yPZBGoDSApQmYOPpl3HLmVew8cyrKM1AaQdKQ1BagtIUbK59DRPPv47N599AaQ5Ke1AahNIilCZha92bmHzxLWy9+DZKs1DahdIwlJahNA3br7yDqVffxfar76E0D6V9KA1EaSFKE7HzxkJMu2mLnTc/QGkmSjtRGorSUpSmYnf9h5h+xwG77yxFaS5Ke1EajNJilCZjb8MyzLznhr33PkJpNkq7URqO0nKUpmP/w1W49ZEO+x8pKM1HaT+KASgWoJiAg41euK3JGwebfFDMQLEDxRAUS1BMweHWAMxpC8ThtiAUc1DsQTEIxSIUk3C0MxTzusJwtCscxSwUu1AMQ7EMxTQc743Ggr4YHO+LRTEPxT4UA1EsRDERxUY0DSWhWIliJoqdkgY9iqUopuLkaDqKsSjWopiLYi+KwSgWo5iMYjMWTeaiWI1iNordMjETiuUopuP080IU41GsRzEfrax24P8BClYoaA=="


def _ax_host():
    import base64
    import zlib
    return np.frombuffer(zlib.decompress(base64.b64decode(_AX_B64)),
                         dtype=np.float32).copy()


def _build():
    nc = bass.Bass()
    img4 = nc.declare_dram_parameter("img4", [B_PER, H, W], f32, isOutput=False)
    tf4 = nc.declare_dram_parameter("tf4", [1, B_PER * 6], f32, isOutput=False)
    axg = nc.declare_dram_parameter("axg", [128, 4], f32, isOutput=False)
    ayg = nc.declare_dram_parameter("ayg", [1, W], f32, isOutput=False)
    zc = nc.declare_dram_parameter("zc", [1, 2056], f32, isOutput=False)
    out4 = nc.declare_dram_parameter("out4", [B_PER, H, W], f32, isOutput=True)
    imgp = nc.dram_tensor("imgp", [B_PER * NP_IMG, 1], f32)
    imgq = nc.dram_tensor("imgq", [B_PER * NPP, 1], f32)

    with tile.TileContext(nc) as tc:
        with (
            tc.tile_pool(name="cst", bufs=1) as cst,
            tc.tile_pool(name="mth", bufs=2) as mth,
            tc.tile_pool(name="gth", bufs=2) as gth,
        ):
            tcall = cst.tile([128, 24], f32)
            axs = cst.tile([128, 4], f32)
            ays = cst.tile([128, W], f32)
            nc.sync.dma_start(out=tcall[:, :], in_=tf4[0:1, :].to_broadcast([128, 24]))
            nc.sync.dma_start(out=axs[:, :], in_=axg[:, :])
            nc.sync.dma_start(out=ays[:, :], in_=ayg[0:1, :].to_broadcast([128, W]))
            # --- stage 0: padded images, then row-pair interleaved images ---
            for b in range(B_PER):
                base = b * NP_IMG
                nc.sync.dma_start(
                    out=bass.AP(imgp, base, [[PW, H], [1, W]]),
                    in_=img4[b, :, :],
                )
                nc.sync.dma_start(  # cols 512..513 zero, rows 0..513
                    out=bass.AP(imgp, base + W, [[PW, PH], [1, 2]]),
                    in_=bass.AP(zc, 0, [[0, PH], [1, 2]]),
                )
                nc.sync.dma_start(  # rows 512..513 zero
                    out=bass.AP(imgp, base + 512 * PW, [[PW, 2], [1, PW]]),
                    in_=bass.AP(zc, 0, [[0, 2], [1, PW]]),
                )
            for b in range(B_PER):
                base = b * NP_IMG
                qbase = b * NPP
                for blk in range(4):
                    r0 = 128 * blk
                    ra = gth.tile([128, PW], f32, tag="ra")
                    rb = gth.tile([128, PW], f32, tag="rb")
                    pr = gth.tile([128, 2 * PW], f32, tag="pr")
                    nc.sync.dma_start(
                        out=ra[:, :],
                        in_=bass.AP(imgp, base + r0 * PW, [[PW, 128], [1, PW]]),
                    )
                    nc.sync.dma_start(
                        out=rb[:, :],
                        in_=bass.AP(imgp, base + (r0 + 1) * PW, [[PW, 128], [1, PW]]),
                    )
                    pr_ap = pr[:, :]
                    pr_ev = bass.AP(pr_ap.tensor, pr_ap.offset, [[2 * PW, 128], [2, PW]])
                    pr_od = bass.AP(pr_ap.tensor, pr_ap.offset + 1, [[2 * PW, 128], [2, PW]])
                    nc.vector.tensor_copy(out=pr_ev, in_=ra[:, :])
                    nc.vector.tensor_copy(out=pr_od, in_=rb[:, :])
                    nc.sync.dma_start(
                        out=bass.AP(imgq, qbase + r0 * PW * 2, [[2 * PW, 128], [1, 2 * PW]]),
                        in_=pr[:, :],
                    )
                # pair-row 512 is all zeros (img rows 512,513 are zero pad);
                # +4 covers the per-image allocation slack (sim cleanliness)
                nc.sync.dma_start(
                    out=bass.AP(imgq, qbase + 512 * PW * 2, [[2056, 1], [1, 1032]]),
                    in_=bass.AP(zc, 0, [[0, 1], [1, 1032]]),
                )

            # --- per (image, 128-row block): coords, weights, gather, blend ---
            for k in range(B_PER * 4):
                b, r = divmod(k, 4)
                m00 = tcall[:, 6 * b + 0 : 6 * b + 1]
                m01 = tcall[:, 6 * b + 1 : 6 * b + 2]
                m10 = tcall[:, 6 * b + 2 : 6 * b + 3]
                m11 = tcall[:, 6 * b + 3 : 6 * b + 4]
                v0 = tcall[:, 6 * b + 4 : 6 * b + 5]
                v1 = tcall[:, 6 * b + 5 : 6 * b + 6]
                axr = axs[:, r : r + 1]

                rx = mth.tile([128, 1], f32, tag="rx")
                ry = mth.tile([128, 1], f32, tag="ry")
                x_t = mth.tile([128, W], f32, tag="x_t")
                y_t = mth.tile([128, W], f32, tag="y_t")
                x0 = mth.tile([128, W], f32, tag="x0")
                y0 = mth.tile([128, W], f32, tag="y0")
                x0c = mth.tile([128, W], f32, tag="x0c")
                x1c = mth.tile([128, W], f32, tag="x1c")
                y0c = mth.tile([128, W], f32, tag="y0c")
                y1c = mth.tile([128, W], f32, tag="y1c")
                fx1 = mth.tile([128, W], f32, tag="fx1")
                fx0 = mth.tile([128, W], f32, tag="fx0")
                fy1 = mth.tile([128, W], f32, tag="fy1")
                fy0 = mth.tile([128, W], f32, tag="fy0")
                w00 = mth.tile([128, W], f32, tag="w00")
                w01 = mth.tile([128, W], f32, tag="w01")
                w10 = mth.tile([128, W], f32, tag="w10")
                w11 = mth.tile([128, W], f32, tag="w11")
                eqt = mth.tile([128, W], f32, tag="eqt")
                tmp = mth.tile([128, W], f32, tag="tmp")
                tm2 = mth.tile([128, W], f32, tag="tm2")
                adA = mth.tile([128, W], f32, tag="adA")
                offa = gth.tile([128, W], i32, tag="offa")
                gall = gth.tile([128, 4 * W], f32, tag="gall")
                acc = gth.tile([128, W], f32, tag="acc")

                nc.vector.tensor_scalar(out=rx[:, :], in0=axr, scalar1=m00,
                                        scalar2=None, op0=A.mult)
                nc.vector.tensor_scalar(out=x_t[:, :], in0=ays[:, :], scalar1=m01,
                                        scalar2=None, op0=A.mult)
                nc.vector.tensor_scalar(out=x_t[:, :], in0=x_t[:, :], scalar1=rx[:, :],
                                        scalar2=v0, op0=A.add, op1=A.add)
                nc.vector.tensor_scalar(out=x_t[:, :], in0=x_t[:, :], scalar1=1.0,
                                        scalar2=0.5, op0=A.add, op1=A.mult)
                nc.vector.tensor_scalar(out=x_t[:, :], in0=x_t[:, :], scalar1=511.0,
                                        scalar2=None, op0=A.mult)
                nc.vector.tensor_scalar(out=x0[:, :], in0=x_t[:, :], scalar1=MAGIC,
                                        scalar2=MAGIC, op0=A.add, op1=A.subtract)
                nc.vector.tensor_scalar(out=x0c[:, :], in0=x0[:, :], scalar1=0.0,
                                        scalar2=512.0, op0=A.max, op1=A.min)
                nc.vector.tensor_scalar(out=x1c[:, :], in0=x0[:, :], scalar1=-1.0,
                                        scalar2=1.0, op0=A.max, op1=A.add)
                nc.vector.tensor_scalar(out=x1c[:, :], in0=x1c[:, :], scalar1=512.0,
                                        scalar2=None, op0=A.min)
                nc.vector.tensor_scalar(out=ry[:, :], in0=axr, scalar1=m10,
                                        scalar2=None, op0=A.mult)
                nc.vector.tensor_scalar(out=y_t[:, :], in0=ays[:, :], scalar1=m11,
                                        scalar2=None, op0=A.mult)
                nc.vector.tensor_scalar(out=y_t[:, :], in0=y_t[:, :], scalar1=ry[:, :],
                                        scalar2=v1, op0=A.add, op1=A.add)
                nc.vector.tensor_scalar(out=y_t[:, :], in0=y_t[:, :], scalar1=1.0,
                                        scalar2=0.5, op0=A.add, op1=A.mult)
                nc.vector.tensor_scalar(out=y_t[:, :], in0=y_t[:, :], scalar1=511.0,
                                        scalar2=None, op0=A.mult)
                nc.vector.tensor_scalar(out=y0[:, :], in0=y_t[:, :], scalar1=MAGIC,
                                        scalar2=MAGIC, op0=A.add, op1=A.subtract)
                nc.vector.tensor_scalar(out=y0c[:, :], in0=y0[:, :], scalar1=0.0,
                                        scalar2=512.0, op0=A.max, op1=A.min)
                nc.vector.tensor_scalar(out=y1c[:, :], in0=y0[:, :], scalar1=-1.0,
                                        scalar2=1.0, op0=A.max, op1=A.add)
                nc.vector.tensor_scalar(out=y1c[:, :], in0=y1c[:, :], scalar1=512.0,
                                        scalar2=None, op0=A.min)
                nc.vector.tensor_tensor(out=fx1[:, :], in0=x1c[:, :], in1=x_t[:, :],
                                        op=A.subtract)
                nc.vector.tensor_tensor(out=fx0[:, :], in0=x_t[:, :], in1=x0c[:, :],
                                        op=A.subtract)
                nc.vector.tensor_tensor(out=fy1[:, :], in0=y1c[:, :], in1=y_t[:, :],
                                        op=A.subtract)
                nc.vector.tensor_tensor(out=fy0[:, :], in0=y_t[:, :], in1=y0c[:, :],
                                        op=A.subtract)
                nc.vector.tensor_tensor(out=w00[:, :], in0=fx1[:, :], in1=fy1[:, :],
                                        op=A.mult)
                nc.vector.tensor_tensor(out=w01[:, :], in0=fx1[:, :], in1=fy0[:, :],
                                        op=A.mult)
                nc.vector.tensor_tensor(out=w10[:, :], in0=fx0[:, :], in1=fy1[:, :],
                                        op=A.mult)
                nc.vector.tensor_tensor(out=w11[:, :], in0=fx0[:, :], in1=fy0[:, :],
                                        op=A.mult)
                # y-fold: where y1c == y0c, odd-column pair elems are wrong
                nc.vector.tensor_tensor(out=eqt[:, :], in0=y0c[:, :], in1=y1c[:, :],
                                        op=A.is_equal)
                nc.vector.tensor_tensor(out=tmp[:, :], in0=eqt[:, :], in1=w01[:, :],
                                        op=A.mult)
                nc.vector.tensor_tensor(out=w00[:, :], in0=w00[:, :], in1=tmp[:, :],
                                        op=A.add)
                nc.vector.tensor_tensor(out=w01[:, :], in0=w01[:, :], in1=tmp[:, :],
                                        op=A.subtract)
                nc.vector.tensor_tensor(out=tm2[:, :], in0=eqt[:, :], in1=w11[:, :],
                                        op=A.mult)
                nc.vector.tensor_tensor(out=w10[:, :], in0=w10[:, :], in1=tm2[:, :],
                                        op=A.add)
                nc.vector.tensor_tensor(out=w11[:, :], in0=w11[:, :], in1=tm2[:, :],
                                        op=A.subtract)
                # x-fold: where x1c == x0c, bottom-row pair elems are wrong
                nc.vector.tensor_tensor(out=eqt[:, :], in0=x0c[:, :], in1=x1c[:, :],
                                        op=A.is_equal)
                nc.vector.tensor_tensor(out=tmp[:, :], in0=eqt[:, :], in1=w10[:, :],
                                        op=A.mult)
                nc.vector.tensor_tensor(out=w00[:, :], in0=w00[:, :], in1=tmp[:, :],
                                        op=A.add)
                nc.vector.tensor_tensor(out=w10[:, :], in0=w10[:, :], in1=tmp[:, :],
                                        op=A.subtract)
                nc.vector.tensor_tensor(out=tm2[:, :], in0=eqt[:, :], in1=w11[:, :],
                                        op=A.mult)
                nc.vector.tensor_tensor(out=w01[:, :], in0=w01[:, :], in1=tm2[:, :],
                                        op=A.add)
                nc.vector.tensor_tensor(out=w11[:, :], in0=w11[:, :], in1=tm2[:, :],
                                        op=A.subtract)
                # pair-image element offset = (x0c*514 + y0c)*2
                nc.vector.scalar_tensor_tensor(out=adA[:, :], in0=x0c[:, :],
                                               scalar=float(PW), in1=y0c[:, :],
                                               op0=A.mult, op1=A.add)
                nc.vector.tensor_scalar(out=adA[:, :], in0=adA[:, :], scalar1=2.0,
                                        scalar2=None, op0=A.mult)
                nc.vector.tensor_copy(out=offa[:, :], in_=adA[:, :])
                # 512 per-pixel gathers: 4 contiguous f32 = all 4 taps
                for j in range(W):
                    nc.gpsimd.indirect_dma_start(
                        out=gall[:, 4 * j : 4 * j + 4], out_offset=None,
                        in_=imgq[:, :],
                        in_offset=bass.IndirectOffsetOnAxis(
                            ap=offa[:, j : j + 1], axis=0),
                        element_offset=b * NPP,
                    )
                # blend: taps v0..v3 = stride-4 views of gall
                g_ap = gall[:, :]
                tv = [bass.AP(g_ap.tensor, g_ap.offset + d, [[4 * W, 128], [4, W]])
                      for d in range(4)]
                nc.vector.tensor_tensor(out=acc[:, :], in0=w00[:, :], in1=tv[0],
                                        op=A.mult)
                nc.vector.tensor_tensor(out=tmp[:, :], in0=w10[:, :], in1=tv[1],
                                        op=A.mult)
                nc.vector.tensor_tensor(out=acc[:, :], in0=acc[:, :], in1=tmp[:, :],
                                        op=A.add)
                nc.vector.tensor_tensor(out=tm2[:, :], in0=w01[:, :], in1=tv[2],
                                        op=A.mult)
                nc.vector.tensor_tensor(out=acc[:, :], in0=acc[:, :], in1=tm2[:, :],
                                        op=A.add)
                nc.vector.tensor_tensor(out=tmp[:, :], in0=w11[:, :], in1=tv[3],
                                        op=A.mult)
                nc.vector.tensor_tensor(out=acc[:, :], in0=acc[:, :], in1=tmp[:, :],
                                        op=A.add)
                nc.sync.dma_start(out=out4[b, 128 * r : 128 * (r + 1), :],
                                  in_=acc[:, :])

    if _LEGALIZE:
        _legalize_multiwaits(nc)
    return nc


def _legalize_multiwaits(nc):
    """This container's walrus cannot encode >1 sem-wait per instruction;
    split extras onto chained wait-NoOps on the same engine."""
    ctr = [0]

    def fresh(engine, wait):
        ctr[0] += 1
        n = mybir.InstNoOp(name=f"I-mwfix-{ctr[0]}", ins=[], outs=[])
        n.engine = engine
        n.sync_info = mybir.SyncInfo(on_wait=[wait], on_update=[])
        n.bass_nofuse = True
        return n

    for fn in nc.m.functions:
        for blk in fn.blocks:
            out = []
            changed = False
            for inst in blk.instructions:
                si = inst.sync_info
                if si is not None and len(si.on_wait) > 1:
                    waits = list(si.on_wait)
                    for w in waits[1:]:
                        out.append(fresh(inst.engine, w))
                    inst.sync_info = mybir.SyncInfo(
                        on_wait=[waits[0]], on_update=list(si.on_update)
                    )
                    changed = True
                out.append(inst)
            if changed:
                blk.instructions = out


_NC = None


def _get_nc():
    global _NC
    if _NC is None:
        _NC = _build()
    return _NC


def kernel(Img, Tform):
    global LAST_EXEC_NS, LAST_RESULTS
    Img = np.ascontiguousarray(np.asarray(Img, dtype=np.float32))
    Tform = np.ascontiguousarray(np.asarray(Tform, dtype=np.float32))
    assert Img.shape == (32, 512, 512, 1) and Tform.shape == (32, 6)

    nc = _get_nc()
    ax = _ax_host()
    axg = np.ascontiguousarray(ax.reshape(4, 128).T)
    ayg = ax.reshape(1, 512).copy()
    zcv = np.zeros((1, 2056), dtype=np.float32)

    in_maps = []
    for k in range(N_CORES):
        sl = slice(B_PER * k, B_PER * (k + 1))
        in_maps.append({
            "img4": np.ascontiguousarray(Img[sl, :, :, 0]),
            "tf4": np.ascontiguousarray(Tform[sl].reshape(1, B_PER * 6)),
            "axg": axg,
            "ayg": ayg,
            "zc": zcv,
        })

    trace = bool(int(os.environ.get("WARP_TRACE", "0")))
    res = run_bass_kernel_spmd(nc, in_maps, list(range(N_CORES)), trace=trace)
    LAST_EXEC_NS = res.exec_time_ns
    LAST_RESULTS = res

    out = np.empty((32, 512, 512, 1), dtype=np.float32)
    for k in range(N_CORES):
        out[B_PER * k : B_PER * (k + 1), :, :, 0] = res.results[k]["out4"]
    return out

